# revision 39
# baseline (speedup 1.0000x reference)
"""ErnieLayout self-attention on 8 Trainium2 NeuronCores (Bass/Tile).

Problem shapes (hardcoded): B=4, S=1024, H=768, NH=12, HD=64.
Sharding: core c -> (batch b = c//2, head-half hh = c%2, i.e. 6 heads).
Each core computes attention for its 6 heads of one batch element and
ships ctx^T per head; the host divides by the softmax denominator and
assembles the [B, S, H] output.

Design (measured ~77us/core vs the 162us v1 baseline; HBM bytes and
per-instruction overheads co-optimized):
  * MASKED-KEY COMPACTION: keys with attention_mask==1 cannot affect
    the output; the host permutes the sequence (unmasked keys first)
    and the kernel streams only kt_eff = max_b ceil(U_b/128) key tiles
    (typically 5 of 8).  The K projection also computes only the
    kt_eff*128 key columns of K^T.
  * REL AS fp16 EXPONENTIALS: softmax(qk + rel1 + rel2) factorizes as
    exp(qk)*exp(rel1+rel2).  The host uploads expRel =
    exp(rel1+rel2 - 4) as fp16 strips in [k, qch, h2, q512] layout
    (4x fewer HBM bytes than the two fp32 rel tensors = the dominant
    stream).  The device computes pT = exp(qk + maskbias) on ACT
    (N=1024 activations from 2-bank PSUM score tiles), then one
    in-place DVE fp16 multiply per (block, qch).  The -4 shift cancels
    in the final division and keeps every fp16 intermediate in range.
  * fp16 uploads for x and W (packed, pre-transposed, Wq and bq
    pre-scaled by 1/8 on the host) - no on-device casts.
  * HOST-SIDE FINALIZE: the device ships ctx^T [65, q] fp16 per head
    (row 64 = the denominator from V's ones column); the host divides,
    transposes, inverse-permutes.  No PE back-transposes/reciprocals.
  * Score tiles pack BOTH heads of a pair side by side per query chunk
    ([:, :512] = head A, [:, 512:] = head B), so the two K=64 QK
    matmuls issue back-to-back with no tile-alloc wait between them and
    run CONCURRENTLY on the PE via row tiling (rows 0-63 / 64-127).
  * FLAT SOFTWARE PIPELINE over the 15 (pair, key-tile) blocks: each
    block emits QK+exp+mul for itself, then the PE-side PV of the
    PREVIOUS block (whose DVE mul finished a block ago), then filler
    projections (d=1,2 q/k groups, V tiles) -- the strict-FIFO PE queue
    never head-of-line-blocks on the current block's exp/mul chain.
  * DMA: all bulk traffic on the sync HWDGE ring in strict consumption
    order (x/wqk_d0 chunks interleaved, wv, rel strips; measured
    ~360-410 GB/s).  SWDGE (gpsimd) only carries tiny inputs + mid
    out-stores (it tops out ~140 GB/s).  PE HAM warmup dummies + an ACT
    exp-table warmup hide the cold-start costs under the prefix DMA.
  * Engine balance: exps on ACT; rel-multiplies, k-copies and v-bias
    adds on DVE; q/proj copies on ACT where the pipeline is PE-bound;
    out copies split ACT/DVE.

Per-core math (identical to reference up to fp16 rounding):
  Q^T = (Wq_s/8 @ X^T + bq/8), K^T = Wk_s @ X^T + bk (fp16 matmuls,
  fp32 PSUM), V = X @ Wv_s^T + bv stored fp16 with a ones column.
  ps[k,q] = K^T.T@Q^T;  pT = exp(ps + maskbias) * expRel[k,q];
  ctx^T[d|1, q] += V_aug[kt].T @ pT[kt];
  host: out[q, h*64+d] = ctx^T[d, q] / ctx^T[64, q].
"""

import os
import sys

import numpy as np

for _p in ("/opt/trn_rl_repo",):
    if _p not in sys.path and os.path.isdir(_p):
        sys.path.append(_p)

import concourse.bass as bass
import concourse.mybir as mybir
import concourse.tile as tile
from concourse import bacc
from concourse.bass_utils import run_bass_kernel_spmd

F32 = mybir.dt.float32
F16 = mybir.dt.float16
I32 = mybir.dt.int32
AF = mybir.ActivationFunctionType
NEG = float(np.finfo(np.float32).min)

P = 128
S = 1024
NH = 6        # heads per core
HD = 64
HIN = 768     # model dim (contraction for projections)
HOUT = NH * HD  # 384, per-core projection width
KT = S // P   # 8 key tiles
VW = HD + 1   # 65: V columns + ones column
NPAIR = NH // 2
SHIFT = 4.0   # exp(s - SHIFT): cancels in the division, tames fp16 range

# engine assignment knobs (tuned on HW)
OUTCOPY = os.environ.get("K_OUTCOPY", "split")   # act | dve | split
PROJCOPY = os.environ.get("K_PROJCOPY", "act")   # engine for d>0 proj copies
NWARM = int(os.environ.get("K_NWARM", "16"))     # PE warmup dummy matmuls
LDWOPT = os.environ.get("K_LDWOPT", "0") == "1"  # walrus LDW-overlap pass

if LDWOPT:
    import concourse.bass_utils as _bu

    _orig_run_command = _bu.run_command

    def _patched_run_command(cmd, *a, **kw):
        cmd = [
            c.replace("--enable-ldw-opt=false", "--enable-ldw-opt=true")
            if isinstance(c, str) else c
            for c in cmd
        ]
        return _orig_run_command(cmd, *a, **kw)

    _bu.run_command = _patched_run_command


def _build_kernel_body(tc, aps, kt_eff):
    import contextlib

    nc = tc.nc
    KTE = kt_eff
    x_ap = aps["x"]          # [128, 6, 1024] f16 (p = hin%128, hc, tok)
    wqk_ap = aps["wqk"]      # [128, 6, 2, 384] f16
    wv_ap = aps["wv"]        # [128, 6, 384] f16
    rel_ap = aps["rel"]      # [NPAIR, KTE, 128, 2048] f16  (k, h2*1024+q)
    mask_ap = aps["mask"]    # [KTE*128] i32
    out_ap = aps["out"]      # [NPAIR, 65, 2048] f16

    with contextlib.ExitStack() as ctx:
        const = ctx.enter_context(tc.tile_pool(name="const", bufs=1))

        # PSUM: score/proj pool 2 x [128,1024] (2 banks each) + ctx^T
        # accumulators 2 x [65,1024] (2 banks each) = 8 banks.
        ps_pool = ctx.enter_context(tc.tile_pool(name="ps", bufs=2, space="PSUM"))
        vpsum = ctx.enter_context(tc.tile_pool(name="vps", bufs=2, space="PSUM"))

        # ACT exp-table warmup: a tiny exp with no DMA dependency so the
        # ~2.7us table load overlaps the initial weight DMAs.
        warm = const.tile([1, 8], F32)
        nc.vector.memset(warm[:], 0.0)
        nc.scalar.activation(warm[:], warm[:], AF.Exp)

        # PE HAM warmup: dummy matmuls with no DMA dependency keep the PE
        # activity window busy during the prefix loads, so the first real
        # projection matmuls run at 2.4 GHz instead of the cold 1.2 GHz.
        if NWARM:
            wt = const.tile([P, 512], F16)
            nc.vector.memset(wt[:], 0.0)
            wps = ps_pool.tile([P, S], F32, tag="ps", name="warmps")
            for i in range(NWARM):
                # alternate halves to avoid PSUM write-after-write drain
                # serialization between consecutive dummies
                half = slice(0, 512) if i % 2 == 0 else slice(512, 1024)
                nc.tensor.matmul(
                    wps[:, half], wt[:, 0:P], wt[:], start=True, stop=True
                )

        # ---------------- input DMAs ------------------------------------
        # sync (HWDGE) ring, strict FIFO in consumption order: x/wqk_d0
        # interleaved by contraction chunk (the d=0 projections start as
        # soon as chunk 0 lands), wv, then the rel strips with the d=1,2
        # weight slices slotted after the first two strips.  The SWDGE
        # (gpsimd) ring only carries the small inputs and the out stores
        # (measured SWDGE tops out ~140 GB/s -- never put the bulk there).
        # Everything bulk goes on ONE ring (sync), interleaved x/wqk_d0
        # chunks first: FIFO order guarantees the prefix is never starved
        # by the strip stream (splitting across rings round-robins the
        # SDMA engines and starves the critical path), and small chunked
        # transfers ramp faster than one big one.
        xa = const.tile([P, 6, S], F16)
        wqk = const.tile([P, 6, 2, P], F16)       # d=0 slices
        wqk2 = const.tile([P, 6, 2, 2 * P], F16)  # d=1,2 slices
        for hc in range(6):
            nc.sync.dma_start(xa[:, hc, :], x_ap[:, hc, :])
            nc.sync.dma_start(wqk[:, hc, :, :], wqk_ap[:, hc, :, 0:P])
        wv = const.tile([P, 6, HOUT], F16)
        nc.sync.dma_start(wv[:], wv_ap[:])

        # gpsimd (SWDGE) ring: mask + biases (tiny)
        mask_i = const.tile([P, KTE], I32)
        nc.gpsimd.dma_start(mask_i[:], mask_ap.rearrange("(a p) -> p a", p=P))
        bias_sb = {}
        for wname in ("q", "k"):
            bt = const.tile([P, 3], F32, tag=f"b{wname}")
            nc.gpsimd.dma_start(
                bt[:], aps[f"b{wname}"].rearrange("(a p) -> p a", p=P)
            )
            bias_sb[wname] = bt
        bv_bc = const.tile([P, NH, HD], F32)
        nc.gpsimd.dma_start(
            bv_bc[:],
            aps["bv"].rearrange("(h d) -> h d", d=HD)[None].to_broadcast(
                (P, NH, HD)
            ),
        )

        r_pool = ctx.enter_context(tc.tile_pool(name="rel", bufs=14))
        strips = [[None] * KTE for _ in range(NPAIR)]

        def emit_strip_dma(dt, kt):
            t = r_pool.tile([P, 2 * S], F16, tag="rel", name=f"r{dt}_{kt}")
            nc.sync.dma_start(t[:], rel_ap[dt, kt])
            strips[dt][kt] = t

        emit_strip_dma(0, 0)
        if KTE > 1:
            emit_strip_dma(0, 1)
        nc.sync.dma_start(wqk2[:], wqk_ap[:, :, :, P:])
        for kt in range(2, KTE):
            emit_strip_dma(0, kt)
        for dt in range(1, NPAIR):
            for kt in range(KTE):
                emit_strip_dma(dt, kt)

        # mask bias: per-partition NEG for masked keys of each kt
        maskb = const.tile([P, KTE], F32)
        nc.vector.tensor_copy(maskb[:], mask_i[:])
        nc.vector.tensor_scalar_mul(maskb[:], maskb[:], NEG)

        # ---------------- long-lived projection outputs -----------------
        qt_pool = ctx.enter_context(tc.tile_pool(name="qT", bufs=3))
        kt_pool = ctx.enter_context(tc.tile_pool(name="kT", bufs=3))
        v_pool = ctx.enter_context(tc.tile_pool(name="v", bufs=KTE))
        qT = [qt_pool.tile([P, S], F16, tag="qT", name=f"qT{i}") for i in range(3)]
        kT = [kt_pool.tile([P, S], F16, tag="kT", name=f"kT{i}") for i in range(3)]
        v_tiles = [
            v_pool.tile([P, NH, VW], F16, tag="v", name=f"v{i}")
            for i in range(KTE)
        ]

        # Keys are compacted: only the first KTE*128 token columns of K^T
        # are ever used, so the K projection skips the rest (~37% of its
        # matmul streams at kt_eff=5).
        KCOLS = min(S, KTE * P)

        def emit_qk_proj(wname, d, cols=None):
            """Projection (sub)group: accumulating matmuls over the 6
            contraction chunks for each 512-col slice of `cols`, + one
            bias-add copy per call.  The d=0 copies run split ACT/DVE in
            the prefix; later ones on ACT (it is starved waiting on the
            PE there, and this keeps the DVE free for the mul->PV chain).
            """
            wi = 0 if wname == "q" else 1
            dest = qT if wname == "q" else kT
            w_sb = wqk if d == 0 else wqk2
            wsl = slice(0, P) if d == 0 else slice((d - 1) * P, d * P)
            if cols is None:
                cols = (0, S if wname == "q" else KCOLS)
            c0, c1 = cols
            pp = ps_pool.tile([P, S], F32, tag="ps", name=f"pp_{wname}{d}")
            for t0 in range(c0, c1, 512):
                t1 = min(t0 + 512, c1)
                for hc in range(6):
                    nc.tensor.matmul(
                        pp[:, t0 - c0:t1 - c0],
                        w_sb[:, hc, wi, wsl],
                        xa[:, hc, t0:t1],
                        start=(hc == 0),
                        stop=(hc == 5),
                    )
            bias_ap = bias_sb[wname][:, d:d + 1]
            if d == 0:
                use_act = wname == "q"
            else:
                use_act = PROJCOPY == "act"
            dsl = dest[d][:, c0:c1]
            psl = pp[:, 0:c1 - c0]
            if use_act:
                nc.scalar.activation(
                    dsl, psl, AF.Identity, bias=bias_ap, scale=1.0
                )
            else:
                nc.vector.tensor_scalar_add(dsl, psl, bias_ap)

        def emit_v_proj(t):
            """V tile t: [128 tok, 6, 65] fp16 with ones column."""
            pv = ps_pool.tile([P, S], F32, tag="ps", name=f"pv{t}")
            for hc in range(6):
                nc.tensor.matmul(
                    pv[:, :HOUT],
                    xa[:, hc, t * P:(t + 1) * P],
                    wv[:, hc, :],
                    start=(hc == 0),
                    stop=(hc == 5),
                )
            nc.vector.memset(v_tiles[t][:, :, HD:HD + 1], 1.0)
            nc.vector.tensor_add(
                v_tiles[t][:, :, 0:HD],
                pv[:, :HOUT].rearrange("p (h d) -> p h d", d=HD),
                bv_bc[:],
            )

        def emit_d0_pair(qcols, kcols):
            """q-d0 and k-d0 half-projections with their matmul chains
            interleaved per contraction chunk: both finish right after the
            last x/w chunk lands instead of serially.  Copies: q on ACT,
            k on DVE (parallel)."""
            ppq = ps_pool.tile([P, S], F32, tag="ps", name=f"ppq{qcols[0]}")
            ppk = (ps_pool.tile([P, S], F32, tag="ps", name=f"ppk{kcols[0]}")
                   if kcols else None)
            for hc in range(6):
                for wi, pp, cols in ((0, ppq, qcols), (1, ppk, kcols)):
                    if not cols:
                        continue
                    c0, c1 = cols
                    for t0 in range(c0, c1, 512):
                        t1 = min(t0 + 512, c1)
                        nc.tensor.matmul(
                            pp[:, t0 - c0:t1 - c0],
                            wqk[:, hc, wi, :],
                            xa[:, hc, t0:t1],
                            start=(hc == 0),
                            stop=(hc == 5),
                        )
            nc.scalar.activation(
                qT[0][:, qcols[0]:qcols[1]],
                ppq[:, 0:qcols[1] - qcols[0]],
                AF.Identity,
                bias=bias_sb["q"][:, 0:1],
                scale=1.0,
            )
            if kcols:
                nc.vector.tensor_scalar_add(
                    kT[0][:, kcols[0]:kcols[1]],
                    ppk[:, 0:kcols[1] - kcols[0]],
                    bias_sb["k"][:, 0:1],
                )

        fillers = [[[] for _ in range(KTE)] for _ in range(NPAIR)]
        for t in range(1, KTE):  # V tile t ready before pair-0 block kt=t
            fillers[0][t - 1].append(lambda t=t: emit_v_proj(t))
        fillers[0][min(2, KTE - 1)].append(lambda: emit_qk_proj("q", 1))
        fillers[0][min(3, KTE - 1)].append(lambda: emit_qk_proj("k", 1))
        fillers[1][min(2, KTE - 1)].append(lambda: emit_qk_proj("q", 2))
        fillers[1][min(3, KTE - 1)].append(lambda: emit_qk_proj("k", 2))

        # ---------------- attention (flat software pipeline) ------------
        # Per flat block: QK+exp+mul for block n, then the PE-side PV of
        # block n-1 (whose mul finished a block ago -> no PE wait), then
        # fillers.  Without the one-block PV lag the strict-FIFO PE queue
        # head-of-line-blocks on the mul of the CURRENT block.
        pt_pool = ctx.enter_context(tc.tile_pool(name="pT", bufs=5))
        out_pool = ctx.enter_context(tc.tile_pool(name="out", bufs=2))

        ctxT_of = {}
        pT_of = {}

        def emit_qk_exp_mul(dt, kt, qchs=(0, 1)):
            # One score tile per query chunk holding BOTH heads side by
            # side: the two QK matmuls are back-to-back with no tile-alloc
            # wait between them, so the PE runs them concurrently via row
            # tiling (contraction rows 0-63 / 64-127).
            if (dt, kt) in pT_of:
                pT = pT_of[(dt, kt)]
            else:
                pT = pt_pool.tile([P, 2 * S], F16, tag="pT",
                                  name=f"pT{dt}_{kt}")
                pT_of[(dt, kt)] = pT
            for qch in qchs:
                ps = ps_pool.tile([P, S], F32, tag="ps",
                                  name=f"s{dt}_{kt}_{qch}")
                qsl = slice(qch * 512, (qch + 1) * 512)
                for h2 in range(2):
                    d0 = h2 * HD
                    nc.tensor.matmul(
                        ps[:, h2 * 512:(h2 + 1) * 512],
                        kT[dt][d0:d0 + HD, kt * P:(kt + 1) * P],
                        qT[dt][d0:d0 + HD, qsl],
                        start=True,
                        stop=True,
                    )
                # exp on ACT (mask as per-partition bias), fp16 out, then
                # one in-place fp16 DVE multiply folds in exp(rel1+rel2-4)
                nc.scalar.activation(
                    pT[:, qch * S:(qch + 1) * S],
                    ps[:],
                    AF.Exp,
                    bias=maskb[:, kt:kt + 1],
                    scale=1.0,
                )
                nc.vector.tensor_mul(
                    pT[:, qch * S:(qch + 1) * S],
                    pT[:, qch * S:(qch + 1) * S],
                    strips[dt][kt][:, qch * S:(qch + 1) * S],
                )

        def emit_pv(dt, kt):
            ctxT = ctxT_of[dt]
            pT = pT_of.pop((dt, kt))
            for qch in range(2):
                qsl = slice(qch * 512, (qch + 1) * 512)
                for h2 in range(2):
                    h = 2 * dt + h2
                    nc.tensor.matmul(
                        ctxT[h2][:, qsl],
                        v_tiles[kt][:, h, :],
                        pT[:, qch * S + h2 * 512:qch * S + (h2 + 1) * 512],
                        start=(kt == 0),
                        stop=(kt == KTE - 1),
                        skip_group_check=True,
                    )

        def emit_out(dt):
            # drain ctx^T to SBUF fp16 and ship; host divides by row 64.
            # Copies run ACT || DVE; stores on the idle SWDGE ring except
            # the last pair (HWDGE ring is empty by then, lower latency,
            # one store per head so the first overlaps the second copy).
            ctxT = ctxT_of.pop(dt)
            ob = out_pool.tile([VW, 2 * S], F16, tag="out", name=f"ob{dt}")
            last = dt == NPAIR - 1
            for h2 in range(2):
                dst = ob[:, h2 * S:(h2 + 1) * S]
                use_act = (OUTCOPY == "act"
                           or ((OUTCOPY == "split" or last) and h2 == 0))
                if use_act:
                    nc.scalar.copy(dst, ctxT[h2][:])
                else:
                    nc.vector.tensor_copy(dst, ctxT[h2][:])
                if last:
                    nc.sync.dma_start(
                        out_ap[dt, :, h2 * S:(h2 + 1) * S], dst
                    )
            if not last:
                nc.gpsimd.dma_start(out_ap[dt], ob[:])

        # Prefix: d0 projections interleaved with the FIRST block's QK at
        # qch granularity, so the first exp fires as soon as the 0:512
        # halves of qT0/kT0 are done.
        ctxT_of[0] = [
            vpsum.tile([VW, S], F32, tag="ctxT", name=f"ctxT0_{h2}")
            for h2 in range(2)
        ]
        emit_d0_pair((0, 512), (0, min(512, KCOLS)))
        emit_qk_exp_mul(0, 0, qchs=(0,))
        emit_d0_pair((512, S), (512, KCOLS) if KCOLS > 512 else None)
        emit_qk_exp_mul(0, 0, qchs=(1,))
        emit_v_proj(0)
        for f in fillers[0][0]:
            f()
        fillers[0][0] = []

        blocks = [(dt, kt) for dt in range(NPAIR) for kt in range(KTE)]
        prev = (0, 0)
        for dt, kt in blocks[1:]:
            if kt == 0:
                ctxT_of[dt] = [
                    vpsum.tile([VW, S], F32, tag="ctxT",
                               name=f"ctxT{dt}_{h2}")
                    for h2 in range(2)
                ]
            emit_qk_exp_mul(dt, kt)
            emit_pv(*prev)
            if prev[1] == KTE - 1:
                emit_out(prev[0])
            for f in fillers[dt][kt]:
                f()
            prev = (dt, kt)
        emit_pv(*prev)
        emit_out(prev[0])


def build_program(kt_eff=8):
    """Build and compile the per-core Bass program. Returns nc."""
    nc = bacc.Bacc(
        "TRN2",
        target_bir_lowering=False,
        debug=False,
        num_devices=8,
    )
    aps = {
        "x": nc.dram_tensor("x", [P, 6, S], F16, kind="ExternalInput").ap(),
        "wqk": nc.dram_tensor("wqk", [P, 6, 2, HOUT], F16, kind="ExternalInput").ap(),
        "wv": nc.dram_tensor("wv", [P, 6, HOUT], F16, kind="ExternalInput").ap(),
        "rel": nc.dram_tensor(
            "rel", [NPAIR, kt_eff, P, 2 * S], F16, kind="ExternalInput"
        ).ap(),
        "mask": nc.dram_tensor("mask", [kt_eff * P], I32, kind="ExternalInput").ap(),
        "bq": nc.dram_tensor("bq", [HOUT], F32, kind="ExternalInput").ap(),
        "bk": nc.dram_tensor("bk", [HOUT], F32, kind="ExternalInput").ap(),
        "bv": nc.dram_tensor("bv", [HOUT], F32, kind="ExternalInput").ap(),
        "out": nc.dram_tensor(
            "out", [NPAIR, VW, 2 * S], F16, kind="ExternalOutput"
        ).ap(),
    }
    with tile.TileContext(nc) as tc:
        _build_kernel_body(tc, aps, kt_eff)
    nc.compile()
    return nc


def make_perms(inputs):
    """Per batch: a sequence permutation putting unmasked keys first, and
    the uniform key-tile count kt_eff = max_b ceil(#unmasked / 128)."""
    am = np.asarray(inputs["attention_mask"]).astype(np.int32)[:, 0, 0, :]
    perms = [np.argsort(am[b], kind="stable") for b in range(4)]
    kt_eff = max(int(-(-int((am[b] == 0).sum()) // P)) for b in range(4))
    kt_eff = max(1, min(KT, kt_eff))
    return perms, kt_eff


def make_in_maps(inputs, perms, kt_eff):
    """Slice/transform full inputs into the 8 per-core input maps."""
    hs = np.asarray(inputs["hidden_states"], np.float32)
    am = np.asarray(inputs["attention_mask"]).astype(np.int32)
    rel1 = np.asarray(inputs["rel_pos"], np.float32)
    rel2 = np.asarray(inputs["rel_2d_pos"], np.float32)
    ws = {k: np.asarray(inputs["W" + k], np.float32) for k in ("q", "k", "v")}
    bs = {k: np.asarray(inputs["b" + k], np.float32) for k in ("q", "k", "v")}

    nk = kt_eff * P
    in_maps = []
    for c in range(8):
        b, hh = divmod(c, 2)
        perm = perms[b]
        kperm = perm[:nk]
        hsl = slice(hh * NH, (hh + 1) * NH)
        csl = slice(hh * HOUT, (hh + 1) * HOUT)

        # expRel strips: exp(rel1+rel2-SHIFT) in transposed ([k, q])
        # permuted layout, fp16, packed [dt, kt, k, qch, h2, q512] to
        # match the device-side pT tile layout.
        r12 = (
            rel1[b, hsl].transpose(0, 2, 1)[:, kperm][:, :, perm]
            + rel2[b, hsl].transpose(0, 2, 1)[:, kperm][:, :, perm]
        )  # [6, nk, 1024] f32
        er = np.exp(r12 - SHIFT).astype(np.float16)  # [6, nk, 1024]
        strips = np.ascontiguousarray(
            er.reshape(NPAIR, 2, kt_eff, P, 2, 512)  # [dt,h2,kt,k,qch,q]
            .transpose(0, 2, 3, 4, 1, 5)             # [dt,kt,k,qch,h2,q]
            .reshape(NPAIR, kt_eff, P, 2 * S)
        )

        # x packed [128, 6, 1024] fp16 (p = hin within chunk, hc, token)
        xp = hs[b].T[:, perm].astype(np.float16)     # [768, 1024]
        x_all = np.ascontiguousarray(
            xp.reshape(6, P, S).transpose(1, 0, 2)
        )

        # W packed fp16, transposed to [hin, out]; Wq pre-scaled by 1/8
        wqT = (ws["q"][csl].T * 0.125).astype(np.float16)  # [768, 384]
        wkT = ws["k"][csl].T.astype(np.float16)
        wvT = ws["v"][csl].T.astype(np.float16)
        wqk_all = np.ascontiguousarray(
            np.stack(
                [wqT.reshape(6, P, HOUT), wkT.reshape(6, P, HOUT)], axis=2
            ).transpose(1, 0, 2, 3)                  # [128, 6, 2, 384]
        )
        wv_all = np.ascontiguousarray(
            wvT.reshape(6, P, HOUT).transpose(1, 0, 2)
        )

        m = {
            "x": x_all,
            "wqk": wqk_all,
            "wv": wv_all,
            "rel": strips,
            "mask": np.ascontiguousarray(am[b, 0, 0][kperm]),
            "bq": np.ascontiguousarray(bs["q"][csl] * 0.125),
            "bk": np.ascontiguousarray(bs["k"][csl]),
            "bv": np.ascontiguousarray(bs["v"][csl]),
        }
        in_maps.append(m)
    return in_maps


def gather_output(results, perms):
    """Divide ctx^T by the denominator row, transpose, inverse-permute."""
    out = np.empty((4, S, HIN), np.float32)
    for c in range(8):
        b, hh = divmod(c, 2)
        r = np.asarray(results[c]["out"], np.float32)  # [NPAIR, 65, 2048]
        r = r.reshape(NPAIR, VW, 2, S)                 # [dt, vw, h2, q]
        ctx = r[:, :HD] / r[:, HD:HD + 1]              # [dt, 64, 2, q]
        # -> [q, dt, h2, d] -> [q, 384]
        blk = ctx.transpose(3, 0, 2, 1).reshape(S, HOUT)
        out[b, perms[b], hh * HOUT:(hh + 1) * HOUT] = blk
    return out


_NC_CACHE = {}


def kernel(**inputs):
    perms, kt_eff = make_perms(inputs)
    if kt_eff not in _NC_CACHE:
        _NC_CACHE[kt_eff] = build_program(kt_eff)
    nc = _NC_CACHE[kt_eff]
    in_maps = make_in_maps(inputs, perms, kt_eff)
    res = run_bass_kernel_spmd(nc, in_maps, list(range(8)))
    return gather_output(res.results, perms)


# revision 40
# speedup vs baseline: 1.2089x; 1.2089x over previous
"""ErnieLayout self-attention on 8 Trainium2 NeuronCores (Bass/Tile).

Problem shapes (hardcoded): B=4, S=1024, H=768, NH=12, HD=64.
Sharding: core c -> (batch b = c//2, head-half hh = c%2, i.e. 6 heads).
Each core computes attention for its 6 heads of one batch element and
ships ctx^T per head; the host divides by the softmax denominator and
assembles the [B, S, H] output.

Design (measured ~77us/core vs the 162us v1 baseline; HBM bytes and
per-instruction overheads co-optimized):
  * MASKED-KEY COMPACTION: keys with attention_mask==1 cannot affect
    the output; the host permutes the sequence (unmasked keys first)
    and the kernel streams only kt_eff = max_b ceil(U_b/128) key tiles
    (typically 5 of 8).  The K projection also computes only the
    kt_eff*128 key columns of K^T.
  * REL AS fp16 EXPONENTIALS: softmax(qk + rel1 + rel2) factorizes as
    exp(qk)*exp(rel1+rel2).  The host uploads expRel =
    exp(rel1+rel2 - 4) as fp16 strips in [k, qch, h2, q512] layout
    (4x fewer HBM bytes than the two fp32 rel tensors = the dominant
    stream).  The device computes pT = exp(qk + maskbias) on ACT
    (N=1024 activations from 2-bank PSUM score tiles), then one
    in-place DVE fp16 multiply per (block, qch).  The -4 shift cancels
    in the final division and keeps every fp16 intermediate in range.
  * fp16 uploads for x and W (packed, pre-transposed, Wq and bq
    pre-scaled by 1/8 on the host) - no on-device casts.
  * HOST-SIDE FINALIZE: the device ships ctx^T [65, q] fp16 per head
    (row 64 = the denominator from V's ones column); the host divides,
    transposes, inverse-permutes.  No PE back-transposes/reciprocals.
  * Score tiles pack BOTH heads of a pair side by side per query chunk
    ([:, :512] = head A, [:, 512:] = head B), so the two K=64 QK
    matmuls issue back-to-back with no tile-alloc wait between them and
    run CONCURRENTLY on the PE via row tiling (rows 0-63 / 64-127).
  * FLAT SOFTWARE PIPELINE over the 15 (pair, key-tile) blocks: each
    block emits QK+exp+mul for itself, then the PE-side PV of the
    PREVIOUS block (whose DVE mul finished a block ago), then filler
    projections (d=1,2 q/k groups, V tiles) -- the strict-FIFO PE queue
    never head-of-line-blocks on the current block's exp/mul chain.
  * DMA: all bulk traffic on the sync HWDGE ring in strict consumption
    order (x/wqk_d0 chunks interleaved, wv, rel strips; measured
    ~360-410 GB/s).  SWDGE (gpsimd) only carries tiny inputs + mid
    out-stores (it tops out ~140 GB/s).  PE HAM warmup dummies + an ACT
    exp-table warmup hide the cold-start costs under the prefix DMA.
  * Engine balance: exps on ACT; rel-multiplies, k-copies and v-bias
    adds on DVE; q/proj copies on ACT where the pipeline is PE-bound;
    out copies split ACT/DVE.

Per-core math (identical to reference up to fp16 rounding):
  Q^T = (Wq_s/8 @ X^T + bq/8), K^T = Wk_s @ X^T + bk (fp16 matmuls,
  fp32 PSUM), V = X @ Wv_s^T + bv stored fp16 with a ones column.
  ps[k,q] = K^T.T@Q^T;  pT = exp(ps + maskbias) * expRel[k,q];
  ctx^T[d|1, q] += V_aug[kt].T @ pT[kt];
  host: out[q, h*64+d] = ctx^T[d, q] / ctx^T[64, q].
"""

import os
import sys

import numpy as np

for _p in ("/opt/trn_rl_repo",):
    if _p not in sys.path and os.path.isdir(_p):
        sys.path.append(_p)

import concourse.bass as bass
import concourse.mybir as mybir
import concourse.tile as tile
from concourse import bacc
from concourse.bass_utils import run_bass_kernel_spmd

F32 = mybir.dt.float32
F16 = mybir.dt.float16
I32 = mybir.dt.int32
AF = mybir.ActivationFunctionType
NEG = float(np.finfo(np.float32).min)

P = 128
S = 1024
NH = 6        # heads per core
HD = 64
HIN = 768     # model dim (contraction for projections)
HOUT = NH * HD  # 384, per-core projection width
KT = S // P   # 8 key tiles
VW = HD + 1   # 65: V columns + ones column
NPAIR = NH // 2
SHIFT = 4.0   # exp(s - SHIFT): cancels in the division, tames fp16 range

# engine assignment knobs (tuned on HW)
OUTCOPY = os.environ.get("K_OUTCOPY", "split")   # act | dve | split
PROJCOPY = os.environ.get("K_PROJCOPY", "act")   # engine for d>0 proj copies
NWARM = int(os.environ.get("K_NWARM", "16"))     # PE warmup dummy matmuls
LDWOPT = os.environ.get("K_LDWOPT", "0") == "1"  # walrus LDW-overlap pass

if LDWOPT:
    import concourse.bass_utils as _bu

    _orig_run_command = _bu.run_command

    def _patched_run_command(cmd, *a, **kw):
        cmd = [
            c.replace("--enable-ldw-opt=false", "--enable-ldw-opt=true")
            if isinstance(c, str) else c
            for c in cmd
        ]
        return _orig_run_command(cmd, *a, **kw)

    _bu.run_command = _patched_run_command


def _build_kernel_body(tc, aps, kt_eff):
    import contextlib

    nc = tc.nc
    KTE = kt_eff
    x_ap = aps["x"]          # [128, 6, 1024] f16 (p = hin%128, hc, tok)
    wqk_ap = aps["wqk"]      # [128, 6, 2, 384] f16
    wv_ap = aps["wv"]        # [128, 6, 384] f16
    rel_ap = aps["rel"]      # [NPAIR, KTE, 128, 2048] f16  (k, h2*1024+q)
    mask_ap = aps["mask"]    # [KTE*128] i32
    out_ap = aps["out"]      # [NPAIR, 65, 2048] f16

    with contextlib.ExitStack() as ctx:
        const = ctx.enter_context(tc.tile_pool(name="const", bufs=1))

        # PSUM: score/proj pool 2 x [128,1024] (2 banks each) + ctx^T
        # accumulators 2 x [65,1024] (2 banks each) = 8 banks.
        ps_pool = ctx.enter_context(tc.tile_pool(name="ps", bufs=2, space="PSUM"))
        vpsum = ctx.enter_context(tc.tile_pool(name="vps", bufs=2, space="PSUM"))

        # ACT exp-table warmup: a tiny exp with no DMA dependency so the
        # ~2.7us table load overlaps the initial weight DMAs.
        warm = const.tile([1, 8], F32)
        nc.vector.memset(warm[:], 0.0)
        nc.scalar.activation(warm[:], warm[:], AF.Exp)

        # PE HAM warmup: dummy matmuls with no DMA dependency keep the PE
        # activity window busy during the prefix loads, so the first real
        # projection matmuls run at 2.4 GHz instead of the cold 1.2 GHz.
        if NWARM:
            wt = const.tile([P, 512], F16)
            nc.vector.memset(wt[:], 0.0)
            wps = ps_pool.tile([P, S], F32, tag="ps", name="warmps")
            for i in range(NWARM):
                # alternate halves to avoid PSUM write-after-write drain
                # serialization between consecutive dummies
                half = slice(0, 512) if i % 2 == 0 else slice(512, 1024)
                nc.tensor.matmul(
                    wps[:, half], wt[:, 0:P], wt[:], start=True, stop=True
                )

        # ---------------- input DMAs ------------------------------------
        # sync (HWDGE) ring, strict FIFO in consumption order: x/wqk_d0
        # interleaved by contraction chunk (the d=0 projections start as
        # soon as chunk 0 lands), wv, then the rel strips with the d=1,2
        # weight slices slotted after the first two strips.  The SWDGE
        # (gpsimd) ring only carries the small inputs and the out stores
        # (measured SWDGE tops out ~140 GB/s -- never put the bulk there).
        # Everything bulk goes on ONE ring (sync), interleaved x/wqk_d0
        # chunks first: FIFO order guarantees the prefix is never starved
        # by the strip stream (splitting across rings round-robins the
        # SDMA engines and starves the critical path), and small chunked
        # transfers ramp faster than one big one.
        xa = const.tile([P, 6, S], F16)
        wqk = const.tile([P, 6, 2, P], F16)       # d=0 slices
        wqk2 = const.tile([P, 6, 2, 2 * P], F16)  # d=1,2 slices
        # one merged w_d0 transfer first (six 64KB chunks run at ~32% DMA
        # efficiency; one 0.39MB transfer doesn't), then x at chunk
        # granularity -- the projection chains are paced by the x chunks
        # either way, and the LAST chunk's landing gates the exp stream.
        nc.sync.dma_start(wqk[:], wqk_ap[:, :, :, 0:P])
        for hc in range(6):
            nc.sync.dma_start(xa[:, hc, :], x_ap[:, hc, :])
        wv = const.tile([P, 6, HOUT], F16)
        nc.sync.dma_start(wv[:], wv_ap[:])

        # gpsimd (SWDGE) ring: mask + biases (tiny)
        mask_i = const.tile([P, KTE], I32)
        nc.gpsimd.dma_start(mask_i[:], mask_ap.rearrange("(a p) -> p a", p=P))
        bias_sb = {}
        for wname in ("q", "k"):
            bt = const.tile([P, 3], F32, tag=f"b{wname}")
            nc.gpsimd.dma_start(
                bt[:], aps[f"b{wname}"].rearrange("(a p) -> p a", p=P)
            )
            bias_sb[wname] = bt
        bv_bc = const.tile([P, NH, HD], F32)
        nc.gpsimd.dma_start(
            bv_bc[:],
            aps["bv"].rearrange("(h d) -> h d", d=HD)[None].to_broadcast(
                (P, NH, HD)
            ),
        )

        r_pool = ctx.enter_context(tc.tile_pool(name="rel", bufs=14))
        strips = [[None] * KTE for _ in range(NPAIR)]

        def emit_strip_dma(dt, kt):
            t = r_pool.tile([P, 2 * S], F16, tag="rel", name=f"r{dt}_{kt}")
            nc.sync.dma_start(t[:], rel_ap[dt, kt])
            strips[dt][kt] = t

        emit_strip_dma(0, 0)
        if KTE > 1:
            emit_strip_dma(0, 1)
        nc.sync.dma_start(wqk2[:], wqk_ap[:, :, :, P:])
        for kt in range(2, KTE):
            emit_strip_dma(0, kt)
        for dt in range(1, NPAIR):
            for kt in range(KTE):
                emit_strip_dma(dt, kt)

        # mask bias: per-partition NEG for masked keys of each kt
        maskb = const.tile([P, KTE], F32)
        nc.vector.tensor_copy(maskb[:], mask_i[:])
        nc.vector.tensor_scalar_mul(maskb[:], maskb[:], NEG)

        # ---------------- long-lived projection outputs -----------------
        qt_pool = ctx.enter_context(tc.tile_pool(name="qT", bufs=3))
        kt_pool = ctx.enter_context(tc.tile_pool(name="kT", bufs=3))
        v_pool = ctx.enter_context(tc.tile_pool(name="v", bufs=KTE))
        qT = [qt_pool.tile([P, S], F16, tag="qT", name=f"qT{i}") for i in range(3)]
        kT = [kt_pool.tile([P, S], F16, tag="kT", name=f"kT{i}") for i in range(3)]
        v_tiles = [
            v_pool.tile([P, NH, VW], F16, tag="v", name=f"v{i}")
            for i in range(KTE)
        ]

        # Keys are compacted: only the first KTE*128 token columns of K^T
        # are ever used, so the K projection skips the rest (~37% of its
        # matmul streams at kt_eff=5).
        KCOLS = min(S, KTE * P)

        def emit_qk_proj(wname, d, cols=None):
            """Projection (sub)group: accumulating matmuls over the 6
            contraction chunks for each 512-col slice of `cols`, + one
            bias-add copy per call.  The d=0 copies run split ACT/DVE in
            the prefix; later ones on ACT (it is starved waiting on the
            PE there, and this keeps the DVE free for the mul->PV chain).
            """
            wi = 0 if wname == "q" else 1
            dest = qT if wname == "q" else kT
            w_sb = wqk if d == 0 else wqk2
            wsl = slice(0, P) if d == 0 else slice((d - 1) * P, d * P)
            if cols is None:
                cols = (0, S if wname == "q" else KCOLS)
            c0, c1 = cols
            pp = ps_pool.tile([P, S], F32, tag="ps", name=f"pp_{wname}{d}")
            for t0 in range(c0, c1, 512):
                t1 = min(t0 + 512, c1)
                for hc in range(6):
                    nc.tensor.matmul(
                        pp[:, t0 - c0:t1 - c0],
                        w_sb[:, hc, wi, wsl],
                        xa[:, hc, t0:t1],
                        start=(hc == 0),
                        stop=(hc == 5),
                    )
            bias_ap = bias_sb[wname][:, d:d + 1]
            if d == 0:
                use_act = wname == "q"
            else:
                use_act = PROJCOPY == "act"
            dsl = dest[d][:, c0:c1]
            psl = pp[:, 0:c1 - c0]
            if use_act:
                nc.scalar.activation(
                    dsl, psl, AF.Identity, bias=bias_ap, scale=1.0
                )
            else:
                nc.vector.tensor_scalar_add(dsl, psl, bias_ap)

        def emit_v_proj(t):
            """V tile t: [128 tok, 6, 65] fp16 with ones column."""
            pv = ps_pool.tile([P, S], F32, tag="ps", name=f"pv{t}")
            for hc in range(6):
                nc.tensor.matmul(
                    pv[:, :HOUT],
                    xa[:, hc, t * P:(t + 1) * P],
                    wv[:, hc, :],
                    start=(hc == 0),
                    stop=(hc == 5),
                )
            nc.vector.memset(v_tiles[t][:, :, HD:HD + 1], 1.0)
            nc.vector.tensor_add(
                v_tiles[t][:, :, 0:HD],
                pv[:, :HOUT].rearrange("p (h d) -> p h d", d=HD),
                bv_bc[:],
            )

        def emit_d0_pair(qcols, kcols):
            """q-d0 and k-d0 half-projections with their matmul chains
            interleaved per contraction chunk: both finish right after the
            last x/w chunk lands instead of serially.  Copies: q on ACT,
            k on DVE (parallel)."""
            ppq = ps_pool.tile([P, S], F32, tag="ps", name=f"ppq{qcols[0]}")
            ppk = (ps_pool.tile([P, S], F32, tag="ps", name=f"ppk{kcols[0]}")
                   if kcols else None)
            for hc in range(6):
                for wi, pp, cols in ((0, ppq, qcols), (1, ppk, kcols)):
                    if not cols:
                        continue
                    c0, c1 = cols
                    for t0 in range(c0, c1, 512):
                        t1 = min(t0 + 512, c1)
                        nc.tensor.matmul(
                            pp[:, t0 - c0:t1 - c0],
                            wqk[:, hc, wi, :],
                            xa[:, hc, t0:t1],
                            start=(hc == 0),
                            stop=(hc == 5),
                        )
            nc.scalar.activation(
                qT[0][:, qcols[0]:qcols[1]],
                ppq[:, 0:qcols[1] - qcols[0]],
                AF.Identity,
                bias=bias_sb["q"][:, 0:1],
                scale=1.0,
            )
            if kcols:
                nc.vector.tensor_scalar_add(
                    kT[0][:, kcols[0]:kcols[1]],
                    ppk[:, 0:kcols[1] - kcols[0]],
                    bias_sb["k"][:, 0:1],
                )

        fillers = [[[] for _ in range(KTE)] for _ in range(NPAIR)]
        for t in range(1, KTE):  # V tile t ready before pair-0 block kt=t
            fillers[0][t - 1].append(lambda t=t: emit_v_proj(t))
        fillers[0][min(2, KTE - 1)].append(lambda: emit_qk_proj("q", 1))
        fillers[0][min(3, KTE - 1)].append(lambda: emit_qk_proj("k", 1))
        fillers[1][min(2, KTE - 1)].append(lambda: emit_qk_proj("q", 2))
        fillers[1][min(3, KTE - 1)].append(lambda: emit_qk_proj("k", 2))

        # ---------------- attention (flat software pipeline) ------------
        # Per flat block: QK+exp+mul for block n, then the PE-side PV of
        # block n-1 (whose mul finished a block ago -> no PE wait), then
        # fillers.  Without the one-block PV lag the strict-FIFO PE queue
        # head-of-line-blocks on the mul of the CURRENT block.
        pt_pool = ctx.enter_context(tc.tile_pool(name="pT", bufs=5))
        out_pool = ctx.enter_context(tc.tile_pool(name="out", bufs=2))

        ctxT_of = {}
        pT_of = {}

        def emit_qk_exp_mul(dt, kt, qchs=(0, 1)):
            # One score tile per query chunk holding BOTH heads side by
            # side: the two QK matmuls are back-to-back with no tile-alloc
            # wait between them, so the PE runs them concurrently via row
            # tiling (contraction rows 0-63 / 64-127).
            if (dt, kt) in pT_of:
                pT = pT_of[(dt, kt)]
            else:
                pT = pt_pool.tile([P, 2 * S], F16, tag="pT",
                                  name=f"pT{dt}_{kt}")
                pT_of[(dt, kt)] = pT
            for qch in qchs:
                ps = ps_pool.tile([P, S], F32, tag="ps",
                                  name=f"s{dt}_{kt}_{qch}")
                qsl = slice(qch * 512, (qch + 1) * 512)
                for h2 in range(2):
                    d0 = h2 * HD
                    nc.tensor.matmul(
                        ps[:, h2 * 512:(h2 + 1) * 512],
                        kT[dt][d0:d0 + HD, kt * P:(kt + 1) * P],
                        qT[dt][d0:d0 + HD, qsl],
                        start=True,
                        stop=True,
                    )
                # exp on ACT (mask as per-partition bias), fp16 out, then
                # one in-place fp16 DVE multiply folds in exp(rel1+rel2-4)
                nc.scalar.activation(
                    pT[:, qch * S:(qch + 1) * S],
                    ps[:],
                    AF.Exp,
                    bias=maskb[:, kt:kt + 1],
                    scale=1.0,
                )
                nc.vector.tensor_mul(
                    pT[:, qch * S:(qch + 1) * S],
                    pT[:, qch * S:(qch + 1) * S],
                    strips[dt][kt][:, qch * S:(qch + 1) * S],
                )

        def emit_pv(dt, kt):
            ctxT = ctxT_of[dt]
            pT = pT_of.pop((dt, kt))
            for qch in range(2):
                qsl = slice(qch * 512, (qch + 1) * 512)
                for h2 in range(2):
                    h = 2 * dt + h2
                    nc.tensor.matmul(
                        ctxT[h2][:, qsl],
                        v_tiles[kt][:, h, :],
                        pT[:, qch * S + h2 * 512:qch * S + (h2 + 1) * 512],
                        start=(kt == 0),
                        stop=(kt == KTE - 1),
                        skip_group_check=True,
                    )

        def emit_out(dt):
            # drain ctx^T to SBUF fp16 and ship; host divides by row 64.
            # Copies run ACT || DVE; stores on the idle SWDGE ring except
            # the last pair (HWDGE ring is empty by then, lower latency,
            # one store per head so the first overlaps the second copy).
            ctxT = ctxT_of.pop(dt)
            ob = out_pool.tile([VW, 2 * S], F16, tag="out", name=f"ob{dt}")
            last = dt == NPAIR - 1
            for h2 in range(2):
                dst = ob[:, h2 * S:(h2 + 1) * S]
                use_act = (OUTCOPY == "act"
                           or ((OUTCOPY == "split" or last) and h2 == 0))
                if use_act:
                    nc.scalar.copy(dst, ctxT[h2][:])
                else:
                    nc.vector.tensor_copy(dst, ctxT[h2][:])
                if last:
                    nc.sync.dma_start(
                        out_ap[dt, :, h2 * S:(h2 + 1) * S], dst
                    )
            if not last:
                nc.gpsimd.dma_start(out_ap[dt], ob[:])

        # Prefix: d0 projections interleaved with the FIRST block's QK at
        # qch granularity, so the first exp fires as soon as the 0:512
        # halves of qT0/kT0 are done.
        ctxT_of[0] = [
            vpsum.tile([VW, S], F32, tag="ctxT", name=f"ctxT0_{h2}")
            for h2 in range(2)
        ]
        emit_d0_pair((0, 512), (0, min(512, KCOLS)))
        emit_qk_exp_mul(0, 0, qchs=(0,))
        emit_d0_pair((512, S), (512, KCOLS) if KCOLS > 512 else None)
        emit_qk_exp_mul(0, 0, qchs=(1,))
        emit_v_proj(0)
        for f in fillers[0][0]:
            f()
        fillers[0][0] = []

        blocks = [(dt, kt) for dt in range(NPAIR) for kt in range(KTE)]
        prev = (0, 0)
        for dt, kt in blocks[1:]:
            if kt == 0:
                ctxT_of[dt] = [
                    vpsum.tile([VW, S], F32, tag="ctxT",
                               name=f"ctxT{dt}_{h2}")
                    for h2 in range(2)
                ]
            emit_qk_exp_mul(dt, kt)
            emit_pv(*prev)
            if prev[1] == KTE - 1:
                emit_out(prev[0])
            for f in fillers[dt][kt]:
                f()
            prev = (dt, kt)
        emit_pv(*prev)
        emit_out(prev[0])


def build_program(kt_eff=8):
    """Build and compile the per-core Bass program. Returns nc."""
    nc = bacc.Bacc(
        "TRN2",
        target_bir_lowering=False,
        debug=False,
        num_devices=8,
    )
    aps = {
        "x": nc.dram_tensor("x", [P, 6, S], F16, kind="ExternalInput").ap(),
        "wqk": nc.dram_tensor("wqk", [P, 6, 2, HOUT], F16, kind="ExternalInput").ap(),
        "wv": nc.dram_tensor("wv", [P, 6, HOUT], F16, kind="ExternalInput").ap(),
        "rel": nc.dram_tensor(
            "rel", [NPAIR, kt_eff, P, 2 * S], F16, kind="ExternalInput"
        ).ap(),
        "mask": nc.dram_tensor("mask", [kt_eff * P], I32, kind="ExternalInput").ap(),
        "bq": nc.dram_tensor("bq", [HOUT], F32, kind="ExternalInput").ap(),
        "bk": nc.dram_tensor("bk", [HOUT], F32, kind="ExternalInput").ap(),
        "bv": nc.dram_tensor("bv", [HOUT], F32, kind="ExternalInput").ap(),
        "out": nc.dram_tensor(
            "out", [NPAIR, VW, 2 * S], F16, kind="ExternalOutput"
        ).ap(),
    }
    with tile.TileContext(nc) as tc:
        _build_kernel_body(tc, aps, kt_eff)
    nc.compile()
    return nc


def make_perms(inputs):
    """Per batch: a sequence permutation putting unmasked keys first, and
    the uniform key-tile count kt_eff = max_b ceil(#unmasked / 128)."""
    am = np.asarray(inputs["attention_mask"]).astype(np.int32)[:, 0, 0, :]
    perms = [np.argsort(am[b], kind="stable") for b in range(4)]
    kt_eff = max(int(-(-int((am[b] == 0).sum()) // P)) for b in range(4))
    kt_eff = max(1, min(KT, kt_eff))
    return perms, kt_eff


def make_in_maps(inputs, perms, kt_eff):
    """Slice/transform full inputs into the 8 per-core input maps."""
    hs = np.asarray(inputs["hidden_states"], np.float32)
    am = np.asarray(inputs["attention_mask"]).astype(np.int32)
    rel1 = np.asarray(inputs["rel_pos"], np.float32)
    rel2 = np.asarray(inputs["rel_2d_pos"], np.float32)
    ws = {k: np.asarray(inputs["W" + k], np.float32) for k in ("q", "k", "v")}
    bs = {k: np.asarray(inputs["b" + k], np.float32) for k in ("q", "k", "v")}

    nk = kt_eff * P
    in_maps = []
    for c in range(8):
        b, hh = divmod(c, 2)
        perm = perms[b]
        kperm = perm[:nk]
        hsl = slice(hh * NH, (hh + 1) * NH)
        csl = slice(hh * HOUT, (hh + 1) * HOUT)

        # expRel strips: exp(rel1+rel2-SHIFT) in transposed ([k, q])
        # permuted layout, fp16, packed [dt, kt, k, qch, h2, q512] to
        # match the device-side pT tile layout.
        r12 = (
            rel1[b, hsl].transpose(0, 2, 1)[:, kperm][:, :, perm]
            + rel2[b, hsl].transpose(0, 2, 1)[:, kperm][:, :, perm]
        )  # [6, nk, 1024] f32
        er = np.exp(r12 - SHIFT).astype(np.float16)  # [6, nk, 1024]
        strips = np.ascontiguousarray(
            er.reshape(NPAIR, 2, kt_eff, P, 2, 512)  # [dt,h2,kt,k,qch,q]
            .transpose(0, 2, 3, 4, 1, 5)             # [dt,kt,k,qch,h2,q]
            .reshape(NPAIR, kt_eff, P, 2 * S)
        )

        # x packed [128, 6, 1024] fp16 (p = hin within chunk, hc, token)
        xp = hs[b].T[:, perm].astype(np.float16)     # [768, 1024]
        x_all = np.ascontiguousarray(
            xp.reshape(6, P, S).transpose(1, 0, 2)
        )

        # W packed fp16, transposed to [hin, out]; Wq pre-scaled by 1/8
        wqT = (ws["q"][csl].T * 0.125).astype(np.float16)  # [768, 384]
        wkT = ws["k"][csl].T.astype(np.float16)
        wvT = ws["v"][csl].T.astype(np.float16)
        wqk_all = np.ascontiguousarray(
            np.stack(
                [wqT.reshape(6, P, HOUT), wkT.reshape(6, P, HOUT)], axis=2
            ).transpose(1, 0, 2, 3)                  # [128, 6, 2, 384]
        )
        wv_all = np.ascontiguousarray(
            wvT.reshape(6, P, HOUT).transpose(1, 0, 2)
        )

        m = {
            "x": x_all,
            "wqk": wqk_all,
            "wv": wv_all,
            "rel": strips,
            "mask": np.ascontiguousarray(am[b, 0, 0][kperm]),
            "bq": np.ascontiguousarray(bs["q"][csl] * 0.125),
            "bk": np.ascontiguousarray(bs["k"][csl]),
            "bv": np.ascontiguousarray(bs["v"][csl]),
        }
        in_maps.append(m)
    return in_maps


def gather_output(results, perms):
    """Divide ctx^T by the denominator row, transpose, inverse-permute."""
    out = np.empty((4, S, HIN), np.float32)
    for c in range(8):
        b, hh = divmod(c, 2)
        r = np.asarray(results[c]["out"], np.float32)  # [NPAIR, 65, 2048]
        r = r.reshape(NPAIR, VW, 2, S)                 # [dt, vw, h2, q]
        ctx = r[:, :HD] / r[:, HD:HD + 1]              # [dt, 64, 2, q]
        # -> [q, dt, h2, d] -> [q, 384]
        blk = ctx.transpose(3, 0, 2, 1).reshape(S, HOUT)
        out[b, perms[b], hh * HOUT:(hh + 1) * HOUT] = blk
    return out


_NC_CACHE = {}


def kernel(**inputs):
    perms, kt_eff = make_perms(inputs)
    if kt_eff not in _NC_CACHE:
        _NC_CACHE[kt_eff] = build_program(kt_eff)
    nc = _NC_CACHE[kt_eff]
    in_maps = make_in_maps(inputs, perms, kt_eff)
    res = run_bass_kernel_spmd(nc, in_maps, list(range(8)))
    return gather_output(res.results, perms)
